# revision 1
# baseline (speedup 1.0000x reference)
"""GCN layer (PyG GCNConv + PReLU) as a Trainium2 Bass kernel, SPMD over 8 NeuronCores.

Math (matching the reference):
    deg[c]  = in_degree(c) + 1          (over edge destinations)
    dis     = deg ** -0.5
    y[s]    = (x[s] * dis[s]) @ W                      # dis-scaled transformed features
    out[c]  = PReLU( dis[c] * sum_{e: col_e = c} y[row_e] + b )
with self-loop edges (i -> i) appended so the self term rides the main path.

Sharding: destinations (output rows) are sharded 12500/core; every core
recomputes the full y locally (cheaper than an all-gather), so cores are
fully independent.  Edges are routed to the core owning their destination,
then binned by (128-dest block, source segment).  Sources are segmented
into 4 ranges of 25088 rows so dma_gather's int16 indices can address y.
Per (block, segment) cell the edges are padded to a fixed tile count; a
single dma_gather per (block-group x segment) fetches y[src] rows for
128-edge tiles ([128 edges, 128 feat] per tile).  A one-hot selection
matrix S[e, d] = (col_rel[e] == d) is built on the vector engine per tile
and S^T @ Y_gathered accumulates over the block's tiles in one PSUM bank
(segment-sum as matmul).  Epilogue applies dis[c], bias, and PReLU.

Host-side prep is limited to sharding/layout: binning + padding edges,
degree counts (a byproduct of binning), folding the diagonal dis scale
into x, and transposing x for the tensor engine's layout.
"""

import math
import numpy as np

P = 128
D = 128


# ----------------------------------------------------------------------------
# Host-side preparation
# ----------------------------------------------------------------------------

def _prep_core(src, col, c, cfg):
    """Per-core edge tables: gather-index sequence (int16, interleaved) and
    per-tile relative destination columns."""
    shard = cfg["shard"]
    NB = cfg["nb"]
    T = cfg["T_cell"]
    NSEG = cfg["n_segs"]
    SEGR = cfg["seg_rows"]
    GB = cfg["grp_blocks"]

    lo = c * shard
    m = (col >= lo) & (col < lo + shard)
    s = src[m].astype(np.int64)
    d = (col[m] - lo).astype(np.int64)
    blk = d >> 7
    seg = s // SEGR
    cell = blk * NSEG + seg
    order = np.argsort(cell, kind="stable")
    s, d, blk, seg, cell = s[order], d[order], blk[order], seg[order], cell[order]
    cnt = np.bincount(cell, minlength=NB * NSEG)
    assert cnt.max() <= T * P, f"cell overflow: {cnt.max()} > {T * P}"
    off = np.concatenate([[0], np.cumsum(cnt)])[:-1]
    r = np.arange(len(s)) - off[cell]
    t = r // P
    p = r % P
    grp = blk // GB
    bi = blk % GB
    # global tile index, ordered (grp, seg, bi, t)
    gt = ((grp * NSEG + seg) * GB + bi) * T + t

    ntiles = NB * NSEG * T
    totidx = ntiles * P
    seq = np.zeros(totidx, np.int16)                 # pad -> row 0 of the segment
    j = gt * P + p
    seq[j] = (s - seg * SEGR).astype(np.int16)
    table16 = np.zeros((16, totidx // 16), np.int16)
    jj = np.arange(totidx)
    table16[jj % 16, jj // 16] = seq
    table = np.tile(table16, (8, 1))                 # replicate across Q7 cores

    crel = np.full((P, ntiles), -1.0, np.float32)    # pad -> matches no dest
    crel[p, gt] = (d - blk * P).astype(np.float32)
    return table, crel


def _host_prep(x, edge_index, W, b, alpha, n_cores):
    x = np.asarray(x, dtype=np.float32)
    ei = np.asarray(edge_index)
    W = np.asarray(W, dtype=np.float32)
    b = np.asarray(b, dtype=np.float32)
    alpha = np.asarray(alpha, dtype=np.float32)
    n_nodes = x.shape[0]
    src, col = ei[0].astype(np.int64), ei[1].astype(np.int64)

    shard = n_nodes // n_cores
    assert shard * n_cores == n_nodes

    deg = (np.bincount(col, minlength=n_nodes) + 1.0).astype(np.float32)
    dis = (1.0 / np.sqrt(deg)).astype(np.float32)

    # self loops ride the main aggregation path
    loops = np.arange(n_nodes, dtype=np.int64)
    src = np.concatenate([src, loops])
    col = np.concatenate([col, loops])

    NSEG = 4
    n_src_pad = ((n_nodes + NSEG * 512 - 1) // (NSEG * 512)) * (NSEG * 512)
    seg_rows = n_src_pad // NSEG
    assert seg_rows <= 32768

    nb_used = math.ceil(shard / P)
    GB = 4
    NB = ((nb_used + GB - 1) // GB) * GB

    # uniform tile count per (block, segment) cell across all cores
    core_of = col // shard
    dloc = col - core_of * shard
    cell_glob = ((core_of * NB) + (dloc >> 7)) * NSEG + (src // seg_rows)
    T_cell = int(math.ceil(
        np.bincount(cell_glob, minlength=n_cores * NB * NSEG).max() / P))

    xT = np.zeros((P, n_src_pad), np.float32)
    xT[:, :n_nodes] = (x * dis[:, None]).T

    iota = np.broadcast_to(np.arange(P, dtype=np.float32), (P, P)).copy()
    alphab = np.broadcast_to(alpha, (P, D)).copy()
    biasb = np.broadcast_to(b, (P, D)).copy()

    cfg = dict(
        n_src_pad=n_src_pad,
        n_groups=n_src_pad // 512,
        nb=NB,
        n_grps=NB // GB,
        grp_blocks=GB,
        T_cell=T_cell,
        n_segs=NSEG,
        seg_rows=seg_rows,
        shard=shard,
        uniform_alpha=bool(np.ptp(alpha) == 0.0),
        alpha0=float(alpha.flat[0]),
        has_bias=bool(np.any(b != 0.0)),
    )

    cores = []
    for c in range(n_cores):
        table, crel = _prep_core(src, col, c, cfg)
        own = np.minimum(c * shard + np.arange(NB * P), n_nodes - 1)
        diso = dis[own.reshape(NB, P).T].astype(np.float32)
        cores.append(dict(gidx=table, crel=crel, diso=diso))

    shared = dict(xT=xT, W=W, iota=iota, alphab=alphab, biasb=biasb)
    return cfg, shared, cores


# ----------------------------------------------------------------------------
# Device program
# ----------------------------------------------------------------------------

def _build_program(cfg):
    import concourse.bass as bass
    import concourse.bacc as bacc
    import concourse.mybir as mybir
    import concourse.tile as tile
    from contextlib import ExitStack

    f32 = mybir.dt.float32
    i16 = mybir.dt.int16
    AF = mybir.ActivationFunctionType
    OP = mybir.AluOpType

    NB = cfg["nb"]
    T = cfg["T_cell"]
    NG = cfg["n_groups"]
    NSP = cfg["n_src_pad"]
    NSEG = cfg["n_segs"]
    SEGR = cfg["seg_rows"]
    GB = cfg["grp_blocks"]
    NGRP = cfg["n_grps"]
    NT_CALL = GB * T                   # tiles per dma_gather call
    CIDX = NT_CALL * P                 # indices per call
    NTILES = NB * NSEG * T
    TOTIDX = NTILES * P

    nc = bacc.Bacc()
    xT = nc.declare_dram_parameter("xT", [P, NSP], f32, isOutput=False)
    Wp = nc.declare_dram_parameter("W", [P, D], f32, isOutput=False)
    gidx = nc.declare_dram_parameter("gidx", [P, TOTIDX // 16], i16, isOutput=False)
    crel = nc.declare_dram_parameter("crel", [P, NTILES], f32, isOutput=False)
    iota = nc.declare_dram_parameter("iota", [P, P], f32, isOutput=False)
    diso = nc.declare_dram_parameter("diso", [P, NB], f32, isOutput=False)
    alphab = nc.declare_dram_parameter("alphab", [P, D], f32, isOutput=False)
    biasb = nc.declare_dram_parameter("biasb", [P, D], f32, isOutput=False)
    out = nc.declare_dram_parameter("out", [NB * P, D], f32, isOutput=True)
    y = nc.dram_tensor("ybuf", [NSP, D], f32)

    with tile.TileContext(nc) as tc, ExitStack() as ctx:
        const_p = ctx.enter_context(tc.tile_pool(name="const", bufs=1))
        W_sb = const_p.tile([P, D], f32)
        nc.sync.dma_start(out=W_sb[:], in_=Wp[:])
        iota_sb = const_p.tile([P, P], f32)
        nc.sync.dma_start(out=iota_sb[:], in_=iota[:])
        diso_sb = const_p.tile([P, NB], f32)
        nc.sync.dma_start(out=diso_sb[:], in_=diso[:])
        alphab_sb = const_p.tile([P, D], f32)
        nc.sync.dma_start(out=alphab_sb[:], in_=alphab[:])
        biasb_sb = const_p.tile([P, D], f32)
        nc.sync.dma_start(out=biasb_sb[:], in_=biasb[:])

        # ---- Phase A: y = xT_pre.T @ W, streamed to DRAM --------------------
        y4 = y[:].rearrange("(g i p) f -> g p i f", i=4, p=P)
        with (
            tc.tile_pool(name="xt", bufs=3) as xt_p,
            tc.tile_pool(name="psA", bufs=2, space="PSUM") as psA_p,
            tc.tile_pool(name="ysb", bufs=3) as y_p,
        ):
            for g in range(NG):
                xt = xt_p.tile([P, 512], f32)
                nc.sync.dma_start(out=xt[:], in_=xT[:][:, g * 512:(g + 1) * 512])
                ps = psA_p.tile([P, 512], f32)
                for i in range(4):
                    nc.tensor.matmul(
                        out=ps[:, i * P:(i + 1) * P],
                        lhsT=xt[:, i * P:(i + 1) * P],
                        rhs=W_sb[:],
                        start=True, stop=True,
                    )
                ysb = y_p.tile([P, 512], f32)
                nc.scalar.activation(ysb[:], ps[:], AF.Copy)
                nc.sync.dma_start(
                    out=y4[g], in_=ysb[:].rearrange("p (i f) -> p i f", i=4)
                )

        # ---- Phase B: gather + one-hot matmul segment-sum per dest block ---
        with (
            tc.tile_pool(name="ix", bufs=2 * NSEG) as ix_p,
            tc.tile_pool(name="crl", bufs=2) as crl_p,
            tc.tile_pool(name="yg", bufs=2 * NSEG) as yg_p,
            tc.tile_pool(name="S", bufs=4) as s_p,
            tc.tile_pool(name="psB", bufs=2, space="PSUM") as psB_p,
            tc.tile_pool(name="eps", bufs=3) as ep_p,
        ):
            for grp in range(NGRP):
                ct = GB * NSEG * T      # crel columns per group
                crl = crl_p.tile([P, ct], f32)
                nc.sync.dma_start(
                    out=crl[:], in_=crel[:][:, grp * ct:(grp + 1) * ct]
                )
                ygs = []
                for seg in range(NSEG):
                    callid = grp * NSEG + seg
                    ix = ix_p.tile([P, CIDX // 16], i16)
                    nc.sync.dma_start(
                        out=ix[:],
                        in_=gidx[:][:, callid * (CIDX // 16):(callid + 1) * (CIDX // 16)],
                    )
                    yg = yg_p.tile([P, NT_CALL * P], f32)
                    nc.gpsimd.dma_gather(
                        out_ap=yg[:].rearrange("p (t f) -> p t f", f=P),
                        in_ap=y[:][seg * SEGR:(seg + 1) * SEGR, :],
                        idxs_ap=ix[:],
                        num_idxs=CIDX,
                        num_idxs_reg=CIDX,
                        elem_size=D,
                        single_packet=False,
                    )
                    ygs.append(yg)
                for bi in range(GB):
                    b2 = grp * GB + bi
                    ps = psB_p.tile([P, P], f32)
                    k = 0
                    for seg in range(NSEG):
                        for t in range(T):
                            S = s_p.tile([P, P], f32)
                            nc.vector.tensor_scalar(
                                S[:], iota_sb[:],
                                crl[:, (seg * GB + bi) * T + t:(seg * GB + bi) * T + t + 1],
                                None, OP.is_equal,
                            )
                            nc.tensor.matmul(
                                out=ps[:], lhsT=S[:],
                                rhs=ygs[seg][:, (bi * T + t) * P:(bi * T + t + 1) * P],
                                start=(k == 0), stop=(k == NSEG * T - 1),
                            )
                            k += 1
                    pre = ep_p.tile([P, P], f32, tag="pre")
                    nc.vector.tensor_scalar(
                        pre[:], ps[:], diso_sb[:, b2:b2 + 1], None, OP.mult
                    )
                    if cfg["has_bias"]:
                        nc.vector.tensor_tensor(
                            out=pre[:], in0=pre[:], in1=biasb_sb[:], op=OP.add
                        )
                    t1 = ep_p.tile([P, P], f32, tag="t1")
                    nc.vector.tensor_scalar(t1[:], pre[:], 0.0, None, OP.max)
                    t2 = ep_p.tile([P, P], f32, tag="t2")
                    if cfg["uniform_alpha"]:
                        nc.vector.tensor_scalar(
                            t2[:], pre[:], 0.0, cfg["alpha0"], OP.min, OP.mult
                        )
                    else:
                        nc.vector.tensor_scalar(t2[:], pre[:], 0.0, None, OP.min)
                        nc.vector.tensor_tensor(
                            out=t2[:], in0=t2[:], in1=alphab_sb[:], op=OP.mult
                        )
                    nc.vector.tensor_tensor(out=t1[:], in0=t1[:], in1=t2[:], op=OP.add)
                    nc.sync.dma_start(
                        out=out[:][b2 * P:(b2 + 1) * P, :], in_=t1[:]
                    )
    nc.finalize()
    return nc


# ----------------------------------------------------------------------------
# Entry point
# ----------------------------------------------------------------------------

N_CORES = 8
TRACE = False          # set True (e.g. from test.py) to capture an NTFF profile
LAST_RESULT = None     # BassKernelResults of the most recent kernel() call


def _install_ntff_hook():
    """Provide antenv.axon_hooks if the image lacks it (needed for trace=True)."""
    import sys, types
    try:
        from antenv import axon_hooks  # noqa: F401
        return
    except ImportError:
        pass
    try:
        import antenv
        from trn_agent_boot.trn_boot import _ntff_profile_via_ctypes
        hook = [_ntff_profile_via_ctypes("/opt/axon/libaxon_pjrt.so")]
    except Exception:
        return
    mod = types.ModuleType("antenv.axon_hooks")
    mod.set_axon_ntff_profile_hook = lambda h: hook.__setitem__(0, h)
    mod.get_axon_ntff_profile_hook = lambda: hook[0]
    sys.modules["antenv.axon_hooks"] = mod
    antenv.axon_hooks = mod


def kernel(x, edge_index, W, b, alpha):
    global LAST_RESULT
    if TRACE:
        _install_ntff_hook()
    from concourse.bass_utils import run_bass_kernel_spmd

    cfg, shared, cores = _host_prep(x, edge_index, W, b, alpha, N_CORES)
    nc = _build_program(cfg)
    in_maps = []
    for c in range(N_CORES):
        m = dict(shared)
        m.update(cores[c])
        in_maps.append(m)
    res = run_bass_kernel_spmd(nc, in_maps, list(range(N_CORES)), trace=TRACE)
    LAST_RESULT = res
    shard = cfg["shard"]
    outs = [np.asarray(res.results[c]["out"])[:shard] for c in range(N_CORES)]
    return np.concatenate(outs, axis=0)



# revision 15
# speedup vs baseline: 1.6422x; 1.6422x over previous
"""GCN layer (PyG GCNConv + PReLU) as a Trainium2 Bass kernel, SPMD over 8 NeuronCores.

Math (matching the reference):
    deg[c]  = in_degree(c) + 1          (over edge destinations)
    dis     = deg ** -0.5
    x_pre   = dis[:,None] * x                           (bf16)
    aggF[:, c] = sum_{e: col_e = c} x_pre[row_e].T  + x_pre[c].T   (features x dests)
    out[c]  = PReLU( dis[c] * (aggF[:, c].T @ W) + b )

Per-core pipeline (dests sharded 12500/core; every core holds the full x_pre):
  * edges binned by (chunk of 7 dest blocks, src segment of 25088 rows), packed
    compactly (dest-sorted, blocks straddle tiles) with TRAILING -1 pads only -
    the dma_gather ucode strips trailing negatives, so padding costs no
    descriptor-generation time (the actual bottleneck: ~8ns/edge on the Pool
    engine's Q7 descriptor-gen loop).
  * one dma_gather per (chunk, seg) cell fetches x_pre[src] rows ([128e, 128f]
    bf16 tiles).
  * one-hot S tiles ([128e, 128d] bf16) for the segment-sum-as-matmul are built
    in ONE DVE instruction per (chunk, seg) via stride-0 broadcast APs:
    S[p, m, j] = (crel[p, m] == iota[j]).
  * per dest block: PSUM accumulates aggF = sum_m Xg_tile(m)^T-free x S(m) over
    the union (static across cores) matmul schedule; a DVE add folds in the
    self-loop term and copies PSUM->SBUF; a second matmul applies W; the DVE
    epilogue applies dis[c], bias, PReLU.
"""

import math
import numpy as np
import ml_dtypes

P = 128
D = 128
BF16 = ml_dtypes.bfloat16


# ----------------------------------------------------------------------------
# Host-side preparation (index/layout only - no per-edge feature data)
# ----------------------------------------------------------------------------

def _host_prep(x, edge_index, W, b, alpha, n_cores):
    x = np.asarray(x, dtype=np.float32)
    ei = np.asarray(edge_index)
    W = np.asarray(W, dtype=np.float32)
    b = np.asarray(b, dtype=np.float32)
    alpha = np.asarray(alpha, dtype=np.float32)
    n_nodes = x.shape[0]
    src, col = ei[0].astype(np.int64), ei[1].astype(np.int64)

    shard = n_nodes // n_cores
    assert shard * n_cores == n_nodes

    deg = (np.bincount(col, minlength=n_nodes) + 1.0).astype(np.float32)
    dis = (1.0 / np.sqrt(deg)).astype(np.float32)

    NSEG = 4
    n_src_pad = ((n_nodes + NSEG * 512 - 1) // (NSEG * 512)) * (NSEG * 512)
    seg_rows = n_src_pad // NSEG
    assert seg_rows <= 32768

    NB = math.ceil(shard / P)          # dest blocks per core (98)
    GB = 7                             # blocks per chunk
    NCH = math.ceil(NB / GB)           # chunks (14)
    NBP = NCH * GB                     # padded block count (98)

    xp = np.zeros((n_src_pad, D), dtype=BF16)
    xp[:n_nodes] = (x * dis[:, None]).astype(BF16)

    iota = np.broadcast_to(
        np.arange(P, dtype=np.float32), (P, P)).astype(BF16).copy()

    # ---- per-core binning ------------------------------------------------
    per_core = []
    T = 0
    for c in range(n_cores):
        lo = c * shard
        m = (col >= lo) & (col < lo + shard)
        s = src[m]
        dloc = col[m] - lo
        bi = dloc >> 7
        ch = bi // GB
        seg = s // seg_rows
        cell = ch * NSEG + seg
        order = np.argsort(cell * NBP + bi, kind="stable")
        s, dloc, bi, ch, seg, cell = (
            s[order], dloc[order], bi[order], ch[order], seg[order], cell[order])
        cnt = np.bincount(cell, minlength=NCH * NSEG)
        off = np.concatenate([[0], np.cumsum(cnt)])[:-1]
        slot = np.arange(len(s)) - off[cell]
        T = max(T, int(math.ceil(cnt.max() / P)))
        per_core.append(dict(s=s, dloc=dloc, bi=bi, cell=cell, slot=slot,
                             cnt=cnt, seg=seg))

    CIDX = T * P

    # ---- union (static) matmul schedule ----------------------------------
    # blockset[(ch, seg, t)] = union over cores of dest blocks present in tile t
    touch = set()
    for pc in per_core:
        cs = pc["cell"]
        t = pc["slot"] >> 7
        k = np.stack([cs // NSEG, cs % NSEG, t, pc["bi"]], axis=1)
        touch.update(map(tuple, np.unique(k, axis=0)))
    colidx = {}
    scol0 = np.zeros((NCH, NSEG), np.int64)    # first col of (ch, seg) S-chunk
    sncol = np.zeros((NCH, NSEG), np.int64)
    ncol = 0
    for ch in range(NCH):
        for seg in range(NSEG):
            scol0[ch, seg] = ncol
            for t in range(T):
                for bb in sorted(bi for (c2, s2, t2, bi) in touch
                                 if c2 == ch and s2 == seg and t2 == t):
                    colidx[(ch, seg, t, bb)] = ncol
                    ncol += 1
            sncol[ch, seg] = ncol - scol0[ch, seg]
    # per-block issue order (seg-major, then tile)
    sched = [[[] for _ in range(GB)] for _ in range(NCH)]
    for (ch, seg, t, bb), colv in sorted(colidx.items(), key=lambda kv: kv[1]):
        sched[ch][bb - ch * GB].append((seg, t, colv))
    for ch in range(NCH):
        for brel in range(GB):
            sched[ch][brel].sort(key=lambda x: (x[0], x[1]))
            assert len(sched[ch][brel]) > 0

    # ---- per-core tables -------------------------------------------------
    cores = []
    for c, pc in enumerate(per_core):
        cell, slot, s, seg = pc["cell"], pc["slot"], pc["s"], pc["seg"]
        # gather indices, wrapped 16 and replicated x8
        seq = np.full(NCH * NSEG * CIDX, -1, np.int16)
        j = cell * CIDX + slot
        seq[j] = (s - seg * seg_rows).astype(np.int16)
        table16 = np.zeros((16, len(seq) // 16), np.int16)
        # within each call the wrap is local: idx j of call q lives at
        # [(j % 16), q*CIDX/16 + j//16]
        q = np.arange(len(seq)) // CIDX
        r = np.arange(len(seq)) % CIDX
        table16[r % 16, q * (CIDX // 16) + r // 16] = seq
        gidx = np.tile(table16, (8, 1))

        crel = np.full((P, ncol), -1.0, np.float32)
        colv = np.array([colidx[(cl // NSEG, cl % NSEG, sl >> 7, bb)]
                         for cl, sl, bb in zip(cell, slot, pc["bi"])],
                        dtype=np.int64)
        crel[slot % P, colv] = (pc["dloc"] - (pc["bi"] << 7)).astype(np.float32)
        crel = crel.astype(BF16)

        own = np.minimum(c * shard + np.arange(NBP * P), n_nodes - 1)
        diso = dis[own.reshape(NBP, P).T].astype(np.float32)
        xpT = np.ascontiguousarray(
            (x[own] * dis[own, None]).T.astype(np.float32))
        cnts_v = pc["cnt"].astype(np.int32).reshape(1, NCH * NSEG)
        cores.append(dict(gidx=gidx, crel=crel, diso=diso, xpT=xpT,
                          cnts=cnts_v))

    cfg = dict(
        n_src_pad=n_src_pad, seg_rows=seg_rows, nb=NB, nbp=NBP, gb=GB,
        nch=NCH, n_segs=NSEG, T=T, cidx=CIDX, ncol=ncol,
        scol0=scol0.tolist(), sncol=sncol.tolist(), sched=sched,
        shard=shard,
        uniform_alpha=bool(np.ptp(alpha) == 0.0),
        alpha0=float(alpha.flat[0]),
        has_bias=bool(np.any(b != 0.0)),
    )
    alphab = np.broadcast_to(alpha, (P, D)).copy()
    biasb = np.broadcast_to(b, (P, D)).copy()
    shared = dict(W=W, iota=iota, alphab=alphab, biasb=biasb)
    for s in range(NSEG):
        shared[f"xp{s}"] = np.ascontiguousarray(
            xp[s * seg_rows:(s + 1) * seg_rows])
    return cfg, shared, cores


# ----------------------------------------------------------------------------
# Device program
# ----------------------------------------------------------------------------

DBG_NCH = None       # debug: limit chunk count
DBG_NOMM = False     # debug: skip matmul chains
DBG_NOGATHER = False  # debug: skip gathers


def _build_program(cfg):
    import concourse.bass as bass
    import concourse.bacc as bacc
    import concourse.mybir as mybir
    import concourse.tile as tile
    from contextlib import ExitStack

    f32 = mybir.dt.float32
    bf16 = mybir.dt.bfloat16
    i16 = mybir.dt.int16
    OP = mybir.AluOpType

    NSP = cfg["n_src_pad"]
    SEGR = cfg["seg_rows"]
    NBP = cfg["nbp"]
    GB = cfg["gb"]
    NCH = cfg["nch"]
    NSEG = cfg["n_segs"]
    T = cfg["T"]
    CIDX = cfg["cidx"]
    NCOL = cfg["ncol"]
    scol0 = cfg["scol0"]
    sncol = cfg["sncol"]
    sched = cfg["sched"]
    TOT16 = NCH * NSEG * (CIDX // 16)

    nc = bacc.Bacc()
    xps = [
        nc.declare_dram_parameter(f"xp{s}", [SEGR, D], bf16, isOutput=False)
        for s in range(NSEG)
    ]
    cnts = nc.declare_dram_parameter("cnts", [1, NCH * NSEG], mybir.dt.int32,
                                     isOutput=False)
    Wp = nc.declare_dram_parameter("W", [P, D], f32, isOutput=False)
    gidx = nc.declare_dram_parameter("gidx", [P, TOT16], i16, isOutput=False)
    crel = nc.declare_dram_parameter("crel", [P, NCOL], bf16, isOutput=False)
    iota = nc.declare_dram_parameter("iota", [P, P], bf16, isOutput=False)
    diso = nc.declare_dram_parameter("diso", [P, NBP], f32, isOutput=False)
    xpT = nc.declare_dram_parameter("xpT", [P, NBP * P], f32, isOutput=False)
    alphab = nc.declare_dram_parameter("alphab", [P, D], f32, isOutput=False)
    biasb = nc.declare_dram_parameter("biasb", [P, D], f32, isOutput=False)
    out = nc.declare_dram_parameter("out", [NBP * P, D], f32, isOutput=True)

    with tile.TileContext(nc) as tc, ExitStack() as ctx:
        const_p = ctx.enter_context(tc.tile_pool(name="const", bufs=1))
        W_sb = const_p.tile([P, D], f32)
        nc.sync.dma_start(out=W_sb[:], in_=Wp[:])
        iota_sb = const_p.tile([P, P], bf16)
        nc.sync.dma_start(out=iota_sb[:], in_=iota[:])
        diso_sb = const_p.tile([P, NBP], f32)
        nc.sync.dma_start(out=diso_sb[:], in_=diso[:])
        alphab_sb = const_p.tile([P, D], f32)
        nc.sync.dma_start(out=alphab_sb[:], in_=alphab[:])
        biasb_sb = const_p.tile([P, D], f32)
        nc.sync.dma_start(out=biasb_sb[:], in_=biasb[:])
        cnt_sb = const_p.tile([1, NCH * NSEG], mybir.dt.int32)
        nc.sync.dma_start(out=cnt_sb[:], in_=cnts[:])
        cnt_reg = nc.gpsimd.alloc_register("gather_cnt")

        with (
            tc.tile_pool(name="ix", bufs=2 * NSEG) as ix_p,
            tc.tile_pool(name="yg", bufs=2 * NSEG) as yg_p,
            tc.tile_pool(name="S", bufs=2 * NSEG) as s_p,
            tc.tile_pool(name="crl", bufs=2) as crl_p,
            tc.tile_pool(name="xpt", bufs=2) as xpt_p,
            tc.tile_pool(name="agg", bufs=3) as agg_p,
            tc.tile_pool(name="psA", bufs=2, space="PSUM") as psA_p,
            tc.tile_pool(name="psB", bufs=2, space="PSUM") as psB_p,
            tc.tile_pool(name="ob", bufs=2) as ob_p,
            tc.tile_pool(name="eps", bufs=3) as ep_p,
        ):
            nch_run = NCH if DBG_NCH is None else DBG_NCH
            for ch in range(nch_run):
                ygs, ss = [], []
                for seg in range(NSEG):
                    q = ch * NSEG + seg
                    ix = ix_p.tile([P, CIDX // 16], i16)
                    nc.sync.dma_start(
                        out=ix[:],
                        in_=gidx[:][:, q * (CIDX // 16):(q + 1) * (CIDX // 16)],
                    )
                    yg = yg_p.tile([P, T * P], bf16)
                    # pad slots are never gathered (trailing -1 stripped by
                    # the ucode); zero the tile so stale/garbage SBUF can't
                    # inject NaN into the masked matmul.
                    nc.vector.memset(yg[:], 0.0)
                    if not DBG_NOGATHER:
                        # num_idxs_reg must equal the per-core count of real
                        # (non-negative) indices: the decode's descriptor-ring
                        # bookkeeping uses the register while the ucode strips
                        # trailing -1 pads; a mismatch desyncs the ring.
                        nc.gpsimd.reg_load(cnt_reg, cnt_sb[0:1, q:q + 1])
                        nc.gpsimd.dma_gather(
                            out_ap=yg[:].rearrange("p (t f) -> p t f", f=P),
                            in_ap=xps[seg][:],
                            idxs_ap=ix[:],
                            num_idxs=CIDX,
                            num_idxs_reg=cnt_reg,
                            elem_size=D,
                            single_packet=False,
                        )
                    ygs.append(yg)
                    nc0 = sncol[ch][seg]
                    crl = crl_p.tile([P, nc0], bf16, tag=f"crl{seg}")
                    nc.sync.dma_start(
                        out=crl[:],
                        in_=crel[:][:, scol0[ch][seg]:scol0[ch][seg] + nc0],
                    )
                    S = s_p.tile([P, nc0 * P], bf16)
                    nc.vector.tensor_tensor(
                        out=S[:].rearrange("p (m j) -> p m j", j=P),
                        in0=crl[:].unsqueeze(2).broadcast_to([P, nc0, P]),
                        in1=iota_sb[:].unsqueeze(1).broadcast_to([P, nc0, P]),
                        op=OP.is_equal,
                    )
                    ss.append(S)

                xpt = xpt_p.tile([P, GB * P], f32)
                nc.sync.dma_start(
                    out=xpt[:], in_=xpT[:][:, ch * GB * P:(ch + 1) * GB * P]
                )
                ob = ob_p.tile([P, GB * D], f32)

                for brel in range(GB):
                    b2 = ch * GB + brel
                    mlist = sched[ch][brel]
                    ps = psA_p.tile([P, P], f32)
                    if DBG_NOMM:
                        mlist = mlist[:1]
                    for k, (seg, t, colv) in enumerate(mlist):
                        lc = colv - scol0[ch][seg]
                        nc.tensor.matmul(
                            out=ps[:],
                            lhsT=ygs[seg][:, t * P:(t + 1) * P],
                            rhs=ss[seg][:, lc * P:(lc + 1) * P],
                            start=(k == 0), stop=(k == len(mlist) - 1),
                        )
                    # fold in self-loop term and move PSUM->SBUF (f32)
                    agg = agg_p.tile([P, P], f32)
                    nc.vector.tensor_tensor(
                        out=agg[:], in0=ps[:],
                        in1=xpt[:, brel * P:(brel + 1) * P], op=OP.add,
                    )
                    ps2 = psB_p.tile([P, D], f32)
                    nc.tensor.matmul(
                        out=ps2[:], lhsT=agg[:], rhs=W_sb[:],
                        start=True, stop=True,
                    )
                    # epilogue: z = dis[c] * ps2 (+ b); prelu
                    pre = ep_p.tile([P, D], f32, tag="pre")
                    nc.vector.tensor_scalar(
                        pre[:], ps2[:], diso_sb[:, b2:b2 + 1], None, OP.mult
                    )
                    if cfg["has_bias"]:
                        nc.vector.tensor_tensor(
                            out=pre[:], in0=pre[:], in1=biasb_sb[:], op=OP.add
                        )
                    t1 = ep_p.tile([P, D], f32, tag="t1")
                    nc.vector.tensor_scalar(t1[:], pre[:], 0.0, None, OP.max)
                    dst = ob[:, brel * D:(brel + 1) * D]
                    if cfg["uniform_alpha"]:
                        t2 = ep_p.tile([P, D], f32, tag="t2")
                        nc.vector.tensor_scalar(
                            t2[:], pre[:], 0.0, cfg["alpha0"], OP.min, OP.mult
                        )
                        nc.vector.tensor_tensor(
                            out=dst, in0=t1[:], in1=t2[:], op=OP.add
                        )
                    else:
                        t2 = ep_p.tile([P, D], f32, tag="t2")
                        nc.vector.tensor_scalar(t2[:], pre[:], 0.0, None, OP.min)
                        nc.vector.tensor_tensor(
                            out=t2[:], in0=t2[:], in1=alphab_sb[:], op=OP.mult
                        )
                        nc.vector.tensor_tensor(
                            out=dst, in0=t1[:], in1=t2[:], op=OP.add
                        )

                nc.sync.dma_start(
                    out=out[:][ch * GB * P:(ch + 1) * GB * P, :]
                    .rearrange("(i p) f -> p i f", p=P),
                    in_=ob[:].rearrange("p (i f) -> p i f", i=GB),
                )
    nc.finalize()
    return nc


# ----------------------------------------------------------------------------
# Entry point
# ----------------------------------------------------------------------------

N_CORES = 8
TRACE = False          # set True (e.g. from test.py) to capture an NTFF profile
LAST_RESULT = None     # BassKernelResults of the most recent kernel() call


def _install_ntff_hook():
    """Provide antenv.axon_hooks if the image lacks it (needed for trace=True)."""
    import sys, types
    try:
        from antenv import axon_hooks  # noqa: F401
        return
    except ImportError:
        pass
    try:
        import antenv
        from trn_agent_boot.trn_boot import _ntff_profile_via_ctypes
        hook = [_ntff_profile_via_ctypes("/opt/axon/libaxon_pjrt.so")]
    except Exception:
        return
    mod = types.ModuleType("antenv.axon_hooks")
    mod.set_axon_ntff_profile_hook = lambda h: hook.__setitem__(0, h)
    mod.get_axon_ntff_profile_hook = lambda: hook[0]
    sys.modules["antenv.axon_hooks"] = mod
    antenv.axon_hooks = mod


def kernel(x, edge_index, W, b, alpha):
    global LAST_RESULT
    if TRACE:
        _install_ntff_hook()
    from concourse.bass_utils import run_bass_kernel_spmd

    cfg, shared, cores = _host_prep(x, edge_index, W, b, alpha, N_CORES)
    nc = _build_program(cfg)
    in_maps = []
    for c in range(N_CORES):
        m = dict(shared)
        m.update(cores[c])
        in_maps.append(m)
    res = run_bass_kernel_spmd(nc, in_maps, list(range(N_CORES)), trace=TRACE)
    LAST_RESULT = res
    shard = cfg["shard"]
    outs = [np.asarray(res.results[c]["out"])[:shard] for c in range(N_CORES)]
    return np.concatenate(outs, axis=0)


# revision 17
# speedup vs baseline: 1.9513x; 1.1882x over previous
"""GCN layer (PyG GCNConv + PReLU) as a Trainium2 Bass kernel, SPMD over 8 NeuronCores.

Math (matching the reference):
    deg[c]  = in_degree(c) + 1          (over edge destinations)
    dis     = deg ** -0.5
    x_pre   = dis[:,None] * x                           (bf16)
    aggF[:, c] = sum_{e: col_e = c} x_pre[row_e].T  + x_pre[c].T   (features x dests)
    out[c]  = PReLU( dis[c] * (aggF[:, c].T @ W) + b )

Per-core pipeline (dests sharded 12500/core; every core holds the full x_pre):
  * edges binned by (chunk of 7 dest blocks, src segment of 25088 rows), packed
    compactly (dest-sorted, blocks straddle tiles) with TRAILING -1 pads only -
    the dma_gather ucode strips trailing negatives, so padding costs no
    descriptor-generation time (the actual bottleneck: ~8ns/edge on the Pool
    engine's Q7 descriptor-gen loop).
  * one dma_gather per (chunk, seg) cell fetches x_pre[src] rows ([128e, 128f]
    bf16 tiles).
  * one-hot S tiles ([128e, 128d] bf16) for the segment-sum-as-matmul are built
    in ONE DVE instruction per (chunk, seg) via stride-0 broadcast APs:
    S[p, m, j] = (crel[p, m] == iota[j]).
  * per dest block: PSUM accumulates aggF = sum_m Xg_tile(m)^T-free x S(m) over
    the union (static across cores) matmul schedule; a DVE add folds in the
    self-loop term and copies PSUM->SBUF; a second matmul applies W; the DVE
    epilogue applies dis[c], bias, PReLU.
"""

import math
import numpy as np
import ml_dtypes

P = 128
D = 128
BF16 = ml_dtypes.bfloat16


# ----------------------------------------------------------------------------
# Host-side preparation (index/layout only - no per-edge feature data)
# ----------------------------------------------------------------------------

def _host_prep(x, edge_index, W, b, alpha, n_cores):
    x = np.asarray(x, dtype=np.float32)
    ei = np.asarray(edge_index)
    W = np.asarray(W, dtype=np.float32)
    b = np.asarray(b, dtype=np.float32)
    alpha = np.asarray(alpha, dtype=np.float32)
    n_nodes = x.shape[0]
    src, col = ei[0].astype(np.int64), ei[1].astype(np.int64)

    shard = n_nodes // n_cores
    assert shard * n_cores == n_nodes

    deg = (np.bincount(col, minlength=n_nodes) + 1.0).astype(np.float32)
    dis = (1.0 / np.sqrt(deg)).astype(np.float32)

    NSEG = 4
    n_src_pad = ((n_nodes + NSEG * 512 - 1) // (NSEG * 512)) * (NSEG * 512)
    seg_rows = n_src_pad // NSEG
    assert seg_rows <= 32768

    NB = math.ceil(shard / P)          # dest blocks per core (98)
    GB = 7                             # blocks per chunk
    NCH = math.ceil(NB / GB)           # chunks (14)
    NBP = NCH * GB                     # padded block count (98)

    xp = np.zeros((n_src_pad, D), dtype=BF16)
    xp[:n_nodes] = (x * dis[:, None]).astype(BF16)

    iota = np.broadcast_to(
        np.arange(P, dtype=np.float32), (P, P)).astype(BF16).copy()

    # ---- per-core binning ------------------------------------------------
    per_core = []
    T = 0
    for c in range(n_cores):
        lo = c * shard
        m = (col >= lo) & (col < lo + shard)
        s = src[m]
        dloc = col[m] - lo
        bi = dloc >> 7
        ch = bi // GB
        seg = s // seg_rows
        cell = ch * NSEG + seg
        order = np.argsort(cell * NBP + bi, kind="stable")
        s, dloc, bi, ch, seg, cell = (
            s[order], dloc[order], bi[order], ch[order], seg[order], cell[order])
        cnt = np.bincount(cell, minlength=NCH * NSEG)
        off = np.concatenate([[0], np.cumsum(cnt)])[:-1]
        slot = np.arange(len(s)) - off[cell]
        T = max(T, int(math.ceil(cnt.max() / P)))
        per_core.append(dict(s=s, dloc=dloc, bi=bi, cell=cell, slot=slot,
                             cnt=cnt, seg=seg))

    CIDX = T * P

    # ---- union (static) matmul schedule ----------------------------------
    # blockset[(ch, seg, t)] = union over cores of dest blocks present in tile t
    touch = set()
    for pc in per_core:
        cs = pc["cell"]
        t = pc["slot"] >> 7
        k = np.stack([cs // NSEG, cs % NSEG, t, pc["bi"]], axis=1)
        touch.update(map(tuple, np.unique(k, axis=0)))
    colidx = {}
    scol0 = np.zeros((NCH, NSEG), np.int64)    # first col of (ch, seg) S-chunk
    sncol = np.zeros((NCH, NSEG), np.int64)
    ncol = 0
    for ch in range(NCH):
        for seg in range(NSEG):
            scol0[ch, seg] = ncol
            for t in range(T):
                for bb in sorted(bi for (c2, s2, t2, bi) in touch
                                 if c2 == ch and s2 == seg and t2 == t):
                    colidx[(ch, seg, t, bb)] = ncol
                    ncol += 1
            sncol[ch, seg] = ncol - scol0[ch, seg]
    # per-block issue order (seg-major, then tile)
    sched = [[[] for _ in range(GB)] for _ in range(NCH)]
    for (ch, seg, t, bb), colv in sorted(colidx.items(), key=lambda kv: kv[1]):
        sched[ch][bb - ch * GB].append((seg, t, colv))
    for ch in range(NCH):
        for brel in range(GB):
            sched[ch][brel].sort(key=lambda x: (x[0], x[1]))
            assert len(sched[ch][brel]) > 0

    # ---- per-core tables -------------------------------------------------
    cores = []
    for c, pc in enumerate(per_core):
        cell, slot, s, seg = pc["cell"], pc["slot"], pc["s"], pc["seg"]
        # gather indices, wrapped 16 and replicated x8
        seq = np.full(NCH * NSEG * CIDX, -1, np.int16)
        j = cell * CIDX + slot
        seq[j] = (s - seg * seg_rows).astype(np.int16)
        table16 = np.zeros((16, len(seq) // 16), np.int16)
        # within each call the wrap is local: idx j of call q lives at
        # [(j % 16), q*CIDX/16 + j//16]
        q = np.arange(len(seq)) // CIDX
        r = np.arange(len(seq)) % CIDX
        table16[r % 16, q * (CIDX // 16) + r // 16] = seq
        gidx = np.tile(table16, (8, 1))

        crel = np.full((P, ncol), -1.0, np.float32)
        colv = np.array([colidx[(cl // NSEG, cl % NSEG, sl >> 7, bb)]
                         for cl, sl, bb in zip(cell, slot, pc["bi"])],
                        dtype=np.int64)
        crel[slot % P, colv] = (pc["dloc"] - (pc["bi"] << 7)).astype(np.float32)
        crel = crel.astype(BF16)

        own = np.minimum(c * shard + np.arange(NBP * P), n_nodes - 1)
        diso = dis[own.reshape(NBP, P).T].astype(np.float32)
        xpT = np.ascontiguousarray(
            (x[own] * dis[own, None]).T.astype(np.float32))
        cnts_v = pc["cnt"].astype(np.int32).reshape(1, NCH * NSEG)
        cores.append(dict(gidx=gidx, crel=crel, diso=diso, xpT=xpT,
                          cnts=cnts_v))

    cfg = dict(
        n_src_pad=n_src_pad, seg_rows=seg_rows, nb=NB, nbp=NBP, gb=GB,
        nch=NCH, n_segs=NSEG, T=T, cidx=CIDX, ncol=ncol,
        scol0=scol0.tolist(), sncol=sncol.tolist(), sched=sched,
        shard=shard,
        uniform_alpha=bool(np.ptp(alpha) == 0.0),
        alpha0=float(alpha.flat[0]),
        has_bias=bool(np.any(b != 0.0)),
    )
    alphab = np.broadcast_to(alpha, (P, D)).copy()
    biasb = np.broadcast_to(b, (P, D)).copy()
    shared = dict(W=W, iota=iota, alphab=alphab, biasb=biasb)
    for s in range(NSEG):
        shared[f"xp{s}"] = np.ascontiguousarray(
            xp[s * seg_rows:(s + 1) * seg_rows])
    return cfg, shared, cores


# ----------------------------------------------------------------------------
# Device program
# ----------------------------------------------------------------------------

DBG_NCH = None       # debug: limit chunk count
DBG_NOMM = False     # debug: skip matmul chains
DBG_NOGATHER = False  # debug: skip gathers


def _build_program(cfg):
    import concourse.bass as bass
    import concourse.bacc as bacc
    import concourse.mybir as mybir
    import concourse.tile as tile
    from contextlib import ExitStack

    f32 = mybir.dt.float32
    bf16 = mybir.dt.bfloat16
    i16 = mybir.dt.int16
    OP = mybir.AluOpType

    NSP = cfg["n_src_pad"]
    SEGR = cfg["seg_rows"]
    NBP = cfg["nbp"]
    GB = cfg["gb"]
    NCH = cfg["nch"]
    NSEG = cfg["n_segs"]
    T = cfg["T"]
    CIDX = cfg["cidx"]
    NCOL = cfg["ncol"]
    scol0 = cfg["scol0"]
    sncol = cfg["sncol"]
    sched = cfg["sched"]
    TOT16 = NCH * NSEG * (CIDX // 16)

    nc = bacc.Bacc(num_swdge_queues=4)
    xps = [
        nc.declare_dram_parameter(f"xp{s}", [SEGR, D], bf16, isOutput=False)
        for s in range(NSEG)
    ]
    cnts = nc.declare_dram_parameter("cnts", [1, NCH * NSEG], mybir.dt.int32,
                                     isOutput=False)
    Wp = nc.declare_dram_parameter("W", [P, D], f32, isOutput=False)
    gidx = nc.declare_dram_parameter("gidx", [P, TOT16], i16, isOutput=False)
    crel = nc.declare_dram_parameter("crel", [P, NCOL], bf16, isOutput=False)
    iota = nc.declare_dram_parameter("iota", [P, P], bf16, isOutput=False)
    diso = nc.declare_dram_parameter("diso", [P, NBP], f32, isOutput=False)
    xpT = nc.declare_dram_parameter("xpT", [P, NBP * P], f32, isOutput=False)
    alphab = nc.declare_dram_parameter("alphab", [P, D], f32, isOutput=False)
    biasb = nc.declare_dram_parameter("biasb", [P, D], f32, isOutput=False)
    out = nc.declare_dram_parameter("out", [NBP * P, D], f32, isOutput=True)

    with tile.TileContext(nc) as tc, ExitStack() as ctx:
        const_p = ctx.enter_context(tc.tile_pool(name="const", bufs=1))
        W_sb = const_p.tile([P, D], f32)
        nc.sync.dma_start(out=W_sb[:], in_=Wp[:])
        iota_sb = const_p.tile([P, P], bf16)
        nc.sync.dma_start(out=iota_sb[:], in_=iota[:])
        diso_sb = const_p.tile([P, NBP], f32)
        nc.sync.dma_start(out=diso_sb[:], in_=diso[:])
        alphab_sb = const_p.tile([P, D], f32)
        nc.sync.dma_start(out=alphab_sb[:], in_=alphab[:])
        biasb_sb = const_p.tile([P, D], f32)
        nc.sync.dma_start(out=biasb_sb[:], in_=biasb[:])
        cnt_sb = const_p.tile([1, NCH * NSEG], mybir.dt.int32)
        nc.sync.dma_start(out=cnt_sb[:], in_=cnts[:])
        cnt_reg = nc.gpsimd.alloc_register("gather_cnt")

        with (
            tc.tile_pool(name="ix", bufs=2 * NSEG) as ix_p,
            tc.tile_pool(name="yg", bufs=2 * NSEG) as yg_p,
            tc.tile_pool(name="S", bufs=2 * NSEG) as s_p,
            tc.tile_pool(name="crl", bufs=2) as crl_p,
            tc.tile_pool(name="xpt", bufs=2) as xpt_p,
            tc.tile_pool(name="agg", bufs=3) as agg_p,
            tc.tile_pool(name="psA", bufs=2, space="PSUM") as psA_p,
            tc.tile_pool(name="psB", bufs=2, space="PSUM") as psB_p,
            tc.tile_pool(name="ob", bufs=2) as ob_p,
            tc.tile_pool(name="eps", bufs=3) as ep_p,
        ):
            nch_run = NCH if DBG_NCH is None else DBG_NCH
            for ch in range(nch_run):
                ygs, ss = [], []
                for seg in range(NSEG):
                    q = ch * NSEG + seg
                    ix = ix_p.tile([P, CIDX // 16], i16)
                    nc.sync.dma_start(
                        out=ix[:],
                        in_=gidx[:][:, q * (CIDX // 16):(q + 1) * (CIDX // 16)],
                    )
                    yg = yg_p.tile([P, T * P], bf16)
                    # pad slots are never gathered (trailing -1 stripped by
                    # the ucode); zero the tile so stale/garbage SBUF can't
                    # inject NaN into the masked matmul.
                    nc.vector.memset(yg[:], 0.0)
                    if not DBG_NOGATHER:
                        # num_idxs_reg must equal the per-core count of real
                        # (non-negative) indices: the decode's descriptor-ring
                        # bookkeeping uses the register while the ucode strips
                        # trailing -1 pads; a mismatch desyncs the ring.
                        nc.gpsimd.reg_load(cnt_reg, cnt_sb[0:1, q:q + 1])
                        nc.gpsimd.dma_gather(
                            out_ap=yg[:].rearrange("p (t f) -> p t f", f=P),
                            in_ap=xps[seg][:],
                            idxs_ap=ix[:],
                            num_idxs=CIDX,
                            num_idxs_reg=cnt_reg,
                            elem_size=D,
                            single_packet=False,
                            queue_num=seg,
                        )
                    ygs.append(yg)
                    nc0 = sncol[ch][seg]
                    crl = crl_p.tile([P, nc0], bf16, tag=f"crl{seg}")
                    nc.sync.dma_start(
                        out=crl[:],
                        in_=crel[:][:, scol0[ch][seg]:scol0[ch][seg] + nc0],
                    )
                    S = s_p.tile([P, nc0 * P], bf16)
                    nc.vector.tensor_tensor(
                        out=S[:].rearrange("p (m j) -> p m j", j=P),
                        in0=crl[:].unsqueeze(2).broadcast_to([P, nc0, P]),
                        in1=iota_sb[:].unsqueeze(1).broadcast_to([P, nc0, P]),
                        op=OP.is_equal,
                    )
                    ss.append(S)

                xpt = xpt_p.tile([P, GB * P], f32)
                nc.sync.dma_start(
                    out=xpt[:], in_=xpT[:][:, ch * GB * P:(ch + 1) * GB * P]
                )
                ob = ob_p.tile([P, GB * D], f32)

                for brel in range(GB):
                    b2 = ch * GB + brel
                    mlist = sched[ch][brel]
                    ps = psA_p.tile([P, P], f32)
                    if DBG_NOMM:
                        mlist = mlist[:1]
                    for k, (seg, t, colv) in enumerate(mlist):
                        lc = colv - scol0[ch][seg]
                        nc.tensor.matmul(
                            out=ps[:],
                            lhsT=ygs[seg][:, t * P:(t + 1) * P],
                            rhs=ss[seg][:, lc * P:(lc + 1) * P],
                            start=(k == 0), stop=(k == len(mlist) - 1),
                        )
                    # fold in self-loop term and move PSUM->SBUF (f32)
                    agg = agg_p.tile([P, P], f32)
                    nc.vector.tensor_tensor(
                        out=agg[:], in0=ps[:],
                        in1=xpt[:, brel * P:(brel + 1) * P], op=OP.add,
                    )
                    ps2 = psB_p.tile([P, D], f32)
                    nc.tensor.matmul(
                        out=ps2[:], lhsT=agg[:], rhs=W_sb[:],
                        start=True, stop=True,
                    )
                    # epilogue: z = dis[c] * ps2 (+ b); prelu
                    pre = ep_p.tile([P, D], f32, tag="pre")
                    nc.vector.tensor_scalar(
                        pre[:], ps2[:], diso_sb[:, b2:b2 + 1], None, OP.mult
                    )
                    if cfg["has_bias"]:
                        nc.vector.tensor_tensor(
                            out=pre[:], in0=pre[:], in1=biasb_sb[:], op=OP.add
                        )
                    t1 = ep_p.tile([P, D], f32, tag="t1")
                    nc.vector.tensor_scalar(t1[:], pre[:], 0.0, None, OP.max)
                    dst = ob[:, brel * D:(brel + 1) * D]
                    if cfg["uniform_alpha"]:
                        t2 = ep_p.tile([P, D], f32, tag="t2")
                        nc.vector.tensor_scalar(
                            t2[:], pre[:], 0.0, cfg["alpha0"], OP.min, OP.mult
                        )
                        nc.vector.tensor_tensor(
                            out=dst, in0=t1[:], in1=t2[:], op=OP.add
                        )
                    else:
                        t2 = ep_p.tile([P, D], f32, tag="t2")
                        nc.vector.tensor_scalar(t2[:], pre[:], 0.0, None, OP.min)
                        nc.vector.tensor_tensor(
                            out=t2[:], in0=t2[:], in1=alphab_sb[:], op=OP.mult
                        )
                        nc.vector.tensor_tensor(
                            out=dst, in0=t1[:], in1=t2[:], op=OP.add
                        )

                nc.sync.dma_start(
                    out=out[:][ch * GB * P:(ch + 1) * GB * P, :]
                    .rearrange("(i p) f -> p i f", p=P),
                    in_=ob[:].rearrange("p (i f) -> p i f", i=GB),
                )
    nc.finalize()
    return nc


# ----------------------------------------------------------------------------
# Entry point
# ----------------------------------------------------------------------------

N_CORES = 8
TRACE = False          # set True (e.g. from test.py) to capture an NTFF profile
LAST_RESULT = None     # BassKernelResults of the most recent kernel() call


def _install_ntff_hook():
    """Provide antenv.axon_hooks if the image lacks it (needed for trace=True)."""
    import sys, types
    try:
        from antenv import axon_hooks  # noqa: F401
        return
    except ImportError:
        pass
    try:
        import antenv
        from trn_agent_boot.trn_boot import _ntff_profile_via_ctypes
        hook = [_ntff_profile_via_ctypes("/opt/axon/libaxon_pjrt.so")]
    except Exception:
        return
    mod = types.ModuleType("antenv.axon_hooks")
    mod.set_axon_ntff_profile_hook = lambda h: hook.__setitem__(0, h)
    mod.get_axon_ntff_profile_hook = lambda: hook[0]
    sys.modules["antenv.axon_hooks"] = mod
    antenv.axon_hooks = mod


def kernel(x, edge_index, W, b, alpha):
    global LAST_RESULT
    if TRACE:
        _install_ntff_hook()
    from concourse.bass_utils import run_bass_kernel_spmd

    cfg, shared, cores = _host_prep(x, edge_index, W, b, alpha, N_CORES)
    nc = _build_program(cfg)
    in_maps = []
    for c in range(N_CORES):
        m = dict(shared)
        m.update(cores[c])
        in_maps.append(m)
    res = run_bass_kernel_spmd(nc, in_maps, list(range(N_CORES)), trace=TRACE)
    LAST_RESULT = res
    shard = cfg["shard"]
    outs = [np.asarray(res.results[c]["out"])[:shard] for c in range(N_CORES)]
    return np.concatenate(outs, axis=0)


# revision 18
# speedup vs baseline: 4.0520x; 2.0766x over previous
"""GCN layer (PyG GCNConv + PReLU) as a Trainium2 Bass kernel, SPMD over 8 NeuronCores.

Math (matching the reference):
    deg[c]  = in_degree(c) + 1          (over edge destinations)
    dis     = deg ** -0.5
    x_pre   = dis[:,None] * x                           (bf16)
    aggF[:, c] = sum_{e: col_e = c} x_pre[row_e].T  + x_pre[c].T   (features x dests)
    out[c]  = PReLU( dis[c] * (aggF[:, c].T @ W) + b )

Per-core pipeline (dests sharded 12500/core; every core holds the full x_pre):
  * edges binned by (chunk of 7 dest blocks, src segment of 25088 rows), packed
    compactly (dest-sorted, blocks straddle tiles) with TRAILING -1 pads only -
    the dma_gather ucode strips trailing negatives, so padding costs no
    descriptor-generation time (the actual bottleneck: ~8ns/edge on the Pool
    engine's Q7 descriptor-gen loop).
  * one dma_gather per (chunk, seg) cell fetches x_pre[src] rows ([128e, 128f]
    bf16 tiles).
  * one-hot S tiles ([128e, 128d] bf16) for the segment-sum-as-matmul are built
    in ONE DVE instruction per (chunk, seg) via stride-0 broadcast APs:
    S[p, m, j] = (crel[p, m] == iota[j]).
  * per dest block: PSUM accumulates aggF = sum_m Xg_tile(m)^T-free x S(m) over
    the union (static across cores) matmul schedule; a DVE add folds in the
    self-loop term and copies PSUM->SBUF; a second matmul applies W; the DVE
    epilogue applies dis[c], bias, PReLU.
"""

import math
import numpy as np
import ml_dtypes

P = 128
D = 128
BF16 = ml_dtypes.bfloat16


# ----------------------------------------------------------------------------
# Host-side preparation (index/layout only - no per-edge feature data)
# ----------------------------------------------------------------------------

def _host_prep(x, edge_index, W, b, alpha, n_cores):
    x = np.asarray(x, dtype=np.float32)
    ei = np.asarray(edge_index)
    W = np.asarray(W, dtype=np.float32)
    b = np.asarray(b, dtype=np.float32)
    alpha = np.asarray(alpha, dtype=np.float32)
    n_nodes = x.shape[0]
    src, col = ei[0].astype(np.int64), ei[1].astype(np.int64)

    shard = n_nodes // n_cores
    assert shard * n_cores == n_nodes

    deg = (np.bincount(col, minlength=n_nodes) + 1.0).astype(np.float32)
    dis = (1.0 / np.sqrt(deg)).astype(np.float32)

    NSEG = 4
    n_src_pad = ((n_nodes + NSEG * 512 - 1) // (NSEG * 512)) * (NSEG * 512)
    seg_rows = n_src_pad // NSEG
    assert seg_rows <= 32768

    NB = math.ceil(shard / P)          # dest blocks per core (98)
    GB = 7                             # blocks per chunk
    NCH = math.ceil(NB / GB)           # chunks (14)
    NBP = NCH * GB                     # padded block count (98)

    xp = np.zeros((n_src_pad, D), dtype=BF16)
    xp[:n_nodes] = (x * dis[:, None]).astype(BF16)

    iota = np.broadcast_to(
        np.arange(P, dtype=np.float32), (P, P)).astype(BF16).copy()

    # ---- per-core binning ------------------------------------------------
    per_core = []
    T = 0
    for c in range(n_cores):
        lo = c * shard
        m = (col >= lo) & (col < lo + shard)
        s = src[m]
        dloc = col[m] - lo
        bi = dloc >> 7
        ch = bi // GB
        seg = s // seg_rows
        cell = ch * NSEG + seg
        order = np.argsort(cell * NBP + bi, kind="stable")
        s, dloc, bi, ch, seg, cell = (
            s[order], dloc[order], bi[order], ch[order], seg[order], cell[order])
        cnt = np.bincount(cell, minlength=NCH * NSEG)
        off = np.concatenate([[0], np.cumsum(cnt)])[:-1]
        slot = np.arange(len(s)) - off[cell]
        T = max(T, int(math.ceil(cnt.max() / P)))
        per_core.append(dict(s=s, dloc=dloc, bi=bi, cell=cell, slot=slot,
                             cnt=cnt, seg=seg))

    CIDX = T * P

    # ---- union (static) matmul schedule ----------------------------------
    # blockset[(ch, seg, t)] = union over cores of dest blocks present in tile t
    touch = set()
    for pc in per_core:
        cs = pc["cell"]
        t = pc["slot"] >> 7
        k = np.stack([cs // NSEG, cs % NSEG, t, pc["bi"]], axis=1)
        touch.update(map(tuple, np.unique(k, axis=0)))
    colidx = {}
    scol0 = np.zeros((NCH, NSEG), np.int64)    # first col of (ch, seg) S-chunk
    sncol = np.zeros((NCH, NSEG), np.int64)
    ncol = 0
    for ch in range(NCH):
        for seg in range(NSEG):
            scol0[ch, seg] = ncol
            for t in range(T):
                for bb in sorted(bi for (c2, s2, t2, bi) in touch
                                 if c2 == ch and s2 == seg and t2 == t):
                    colidx[(ch, seg, t, bb)] = ncol
                    ncol += 1
            sncol[ch, seg] = ncol - scol0[ch, seg]
    # per-block issue order (seg-major, then tile)
    sched = [[[] for _ in range(GB)] for _ in range(NCH)]
    for (ch, seg, t, bb), colv in sorted(colidx.items(), key=lambda kv: kv[1]):
        sched[ch][bb - ch * GB].append((seg, t, colv))
    for ch in range(NCH):
        for brel in range(GB):
            sched[ch][brel].sort(key=lambda x: (x[0], x[1]))
            assert len(sched[ch][brel]) > 0

    # ---- per-core tables -------------------------------------------------
    cores = []
    for c, pc in enumerate(per_core):
        cell, slot, s, seg = pc["cell"], pc["slot"], pc["s"], pc["seg"]
        # gather indices, wrapped 16 and replicated x8
        seq = np.full(NCH * NSEG * CIDX, -1, np.int16)
        j = cell * CIDX + slot
        seq[j] = (s - seg * seg_rows).astype(np.int16)
        table16 = np.zeros((16, len(seq) // 16), np.int16)
        # within each call the wrap is local: idx j of call q lives at
        # [(j % 16), q*CIDX/16 + j//16]
        q = np.arange(len(seq)) // CIDX
        r = np.arange(len(seq)) % CIDX
        table16[r % 16, q * (CIDX // 16) + r // 16] = seq
        gidx = np.tile(table16, (8, 1))

        crel = np.full((P, ncol), -1.0, np.float32)
        colv = np.array([colidx[(cl // NSEG, cl % NSEG, sl >> 7, bb)]
                         for cl, sl, bb in zip(cell, slot, pc["bi"])],
                        dtype=np.int64)
        crel[slot % P, colv] = (pc["dloc"] - (pc["bi"] << 7)).astype(np.float32)
        crel = crel.astype(BF16)

        own = np.minimum(c * shard + np.arange(NBP * P), n_nodes - 1)
        diso = dis[own.reshape(NBP, P).T].astype(np.float32)
        xpT = np.ascontiguousarray(
            (x[own] * dis[own, None]).T.astype(np.float32))
        cnts_v = pc["cnt"].astype(np.int32).reshape(1, NCH * NSEG)
        cores.append(dict(gidx=gidx, crel=crel, diso=diso, xpT=xpT,
                          cnts=cnts_v))

    cfg = dict(
        n_src_pad=n_src_pad, seg_rows=seg_rows, nb=NB, nbp=NBP, gb=GB,
        nch=NCH, n_segs=NSEG, T=T, cidx=CIDX, ncol=ncol,
        scol0=scol0.tolist(), sncol=sncol.tolist(), sched=sched,
        shard=shard,
        uniform_alpha=bool(np.ptp(alpha) == 0.0),
        alpha0=float(alpha.flat[0]),
        has_bias=bool(np.any(b != 0.0)),
    )
    alphab = np.broadcast_to(alpha, (P, D)).copy()
    biasb = np.broadcast_to(b, (P, D)).copy()
    shared = dict(W=W, iota=iota, alphab=alphab, biasb=biasb)
    for s in range(NSEG):
        shared[f"xp{s}"] = np.ascontiguousarray(
            xp[s * seg_rows:(s + 1) * seg_rows])
    return cfg, shared, cores


# ----------------------------------------------------------------------------
# Device program
# ----------------------------------------------------------------------------

DBG_NCH = None       # debug: limit chunk count
DBG_NOMM = False     # debug: skip matmul chains
DBG_NOGATHER = False  # debug: skip gathers


def _build_program(cfg):
    import concourse.bass as bass
    import concourse.bacc as bacc
    import concourse.mybir as mybir
    import concourse.tile as tile
    from contextlib import ExitStack

    f32 = mybir.dt.float32
    bf16 = mybir.dt.bfloat16
    i16 = mybir.dt.int16
    OP = mybir.AluOpType

    NSP = cfg["n_src_pad"]
    SEGR = cfg["seg_rows"]
    NBP = cfg["nbp"]
    GB = cfg["gb"]
    NCH = cfg["nch"]
    NSEG = cfg["n_segs"]
    T = cfg["T"]
    CIDX = cfg["cidx"]
    NCOL = cfg["ncol"]
    scol0 = cfg["scol0"]
    sncol = cfg["sncol"]
    sched = cfg["sched"]
    TOT16 = NCH * NSEG * (CIDX // 16)

    nc = bacc.Bacc(num_swdge_queues=4)
    xps = [
        nc.declare_dram_parameter(f"xp{s}", [SEGR, D], bf16, isOutput=False)
        for s in range(NSEG)
    ]
    cnts = nc.declare_dram_parameter("cnts", [1, NCH * NSEG], mybir.dt.int32,
                                     isOutput=False)
    Wp = nc.declare_dram_parameter("W", [P, D], f32, isOutput=False)
    gidx = nc.declare_dram_parameter("gidx", [P, TOT16], i16, isOutput=False)
    crel = nc.declare_dram_parameter("crel", [P, NCOL], bf16, isOutput=False)
    iota = nc.declare_dram_parameter("iota", [P, P], bf16, isOutput=False)
    diso = nc.declare_dram_parameter("diso", [P, NBP], f32, isOutput=False)
    xpT = nc.declare_dram_parameter("xpT", [P, NBP * P], f32, isOutput=False)
    alphab = nc.declare_dram_parameter("alphab", [P, D], f32, isOutput=False)
    biasb = nc.declare_dram_parameter("biasb", [P, D], f32, isOutput=False)
    out = nc.declare_dram_parameter("out", [NBP * P, D], f32, isOutput=True)

    with tile.TileContext(nc) as tc, ExitStack() as ctx:
        const_p = ctx.enter_context(tc.tile_pool(name="const", bufs=1))
        W_sb = const_p.tile([P, D], f32)
        nc.sync.dma_start(out=W_sb[:], in_=Wp[:])
        iota_sb = const_p.tile([P, P], bf16)
        nc.sync.dma_start(out=iota_sb[:], in_=iota[:])
        diso_sb = const_p.tile([P, NBP], f32)
        nc.sync.dma_start(out=diso_sb[:], in_=diso[:])
        alphab_sb = const_p.tile([P, D], f32)
        nc.sync.dma_start(out=alphab_sb[:], in_=alphab[:])
        biasb_sb = const_p.tile([P, D], f32)
        nc.sync.dma_start(out=biasb_sb[:], in_=biasb[:])
        cnt_sb = const_p.tile([1, NCH * NSEG], mybir.dt.int32)
        nc.sync.dma_start(out=cnt_sb[:], in_=cnts[:])
        cnt_reg = nc.gpsimd.alloc_register("gather_cnt")

        with (
            tc.tile_pool(name="ix", bufs=2 * NSEG) as ix_p,
            tc.tile_pool(name="yg", bufs=2 * NSEG) as yg_p,
            tc.tile_pool(name="S", bufs=2 * NSEG) as s_p,
            tc.tile_pool(name="crl", bufs=2) as crl_p,
            tc.tile_pool(name="xpt", bufs=2) as xpt_p,
            tc.tile_pool(name="agg", bufs=3) as agg_p,
            tc.tile_pool(name="psA", bufs=2, space="PSUM") as psA_p,
            tc.tile_pool(name="psB", bufs=2, space="PSUM") as psB_p,
            tc.tile_pool(name="ob", bufs=2) as ob_p,
            tc.tile_pool(name="eps", bufs=3) as ep_p,
        ):
            nch_run = NCH if DBG_NCH is None else DBG_NCH
            for ch in range(nch_run):
                ygs, ss = [], []
                for seg in range(NSEG):
                    q = ch * NSEG + seg
                    ix = ix_p.tile([P, CIDX // 16], i16)
                    nc.sync.dma_start(
                        out=ix[:],
                        in_=gidx[:][:, q * (CIDX // 16):(q + 1) * (CIDX // 16)],
                    )
                    yg = yg_p.tile([P, T * P], bf16)
                    # Pad slots are never gathered (trailing -1 stripped by
                    # the ucode); they are masked by S==0 in the matmul, which
                    # is only NaN-safe if the stale bits are finite. Virgin
                    # SBUF isn't guaranteed finite, so zero each buffer on its
                    # first two uses; after that the slots hold old gathered
                    # x values (finite), and skipping the memset removes a
                    # DVE->Pool dependency that stalls the gather queue.
                    if ch < 2:
                        nc.vector.memset(yg[:], 0.0)
                    if not DBG_NOGATHER:
                        # num_idxs_reg must equal the per-core count of real
                        # (non-negative) indices: the decode's descriptor-ring
                        # bookkeeping uses the register while the ucode strips
                        # trailing -1 pads; a mismatch desyncs the ring.
                        nc.gpsimd.reg_load(cnt_reg, cnt_sb[0:1, q:q + 1])
                        nc.gpsimd.dma_gather(
                            out_ap=yg[:].rearrange("p (t f) -> p t f", f=P),
                            in_ap=xps[seg][:],
                            idxs_ap=ix[:],
                            num_idxs=CIDX,
                            num_idxs_reg=cnt_reg,
                            elem_size=D,
                            single_packet=False,
                            queue_num=seg,
                        )
                    ygs.append(yg)
                    nc0 = sncol[ch][seg]
                    crl = crl_p.tile([P, nc0], bf16, tag=f"crl{seg}")
                    nc.sync.dma_start(
                        out=crl[:],
                        in_=crel[:][:, scol0[ch][seg]:scol0[ch][seg] + nc0],
                    )
                    S = s_p.tile([P, nc0 * P], bf16)
                    nc.vector.tensor_tensor(
                        out=S[:].rearrange("p (m j) -> p m j", j=P),
                        in0=crl[:].unsqueeze(2).broadcast_to([P, nc0, P]),
                        in1=iota_sb[:].unsqueeze(1).broadcast_to([P, nc0, P]),
                        op=OP.is_equal,
                    )
                    ss.append(S)

                xpt = xpt_p.tile([P, GB * P], f32)
                nc.sync.dma_start(
                    out=xpt[:], in_=xpT[:][:, ch * GB * P:(ch + 1) * GB * P]
                )
                ob = ob_p.tile([P, GB * D], f32)

                for brel in range(GB):
                    b2 = ch * GB + brel
                    mlist = sched[ch][brel]
                    ps = psA_p.tile([P, P], f32)
                    if DBG_NOMM:
                        mlist = mlist[:1]
                    for k, (seg, t, colv) in enumerate(mlist):
                        lc = colv - scol0[ch][seg]
                        nc.tensor.matmul(
                            out=ps[:],
                            lhsT=ygs[seg][:, t * P:(t + 1) * P],
                            rhs=ss[seg][:, lc * P:(lc + 1) * P],
                            start=(k == 0), stop=(k == len(mlist) - 1),
                        )
                    # fold in self-loop term and move PSUM->SBUF (f32)
                    agg = agg_p.tile([P, P], f32)
                    nc.vector.tensor_tensor(
                        out=agg[:], in0=ps[:],
                        in1=xpt[:, brel * P:(brel + 1) * P], op=OP.add,
                    )
                    ps2 = psB_p.tile([P, D], f32)
                    nc.tensor.matmul(
                        out=ps2[:], lhsT=agg[:], rhs=W_sb[:],
                        start=True, stop=True,
                    )
                    # epilogue: z = dis[c] * ps2 (+ b); prelu
                    pre = ep_p.tile([P, D], f32, tag="pre")
                    nc.vector.tensor_scalar(
                        pre[:], ps2[:], diso_sb[:, b2:b2 + 1], None, OP.mult
                    )
                    if cfg["has_bias"]:
                        nc.vector.tensor_tensor(
                            out=pre[:], in0=pre[:], in1=biasb_sb[:], op=OP.add
                        )
                    t1 = ep_p.tile([P, D], f32, tag="t1")
                    nc.vector.tensor_scalar(t1[:], pre[:], 0.0, None, OP.max)
                    dst = ob[:, brel * D:(brel + 1) * D]
                    if cfg["uniform_alpha"]:
                        t2 = ep_p.tile([P, D], f32, tag="t2")
                        nc.vector.tensor_scalar(
                            t2[:], pre[:], 0.0, cfg["alpha0"], OP.min, OP.mult
                        )
                        nc.vector.tensor_tensor(
                            out=dst, in0=t1[:], in1=t2[:], op=OP.add
                        )
                    else:
                        t2 = ep_p.tile([P, D], f32, tag="t2")
                        nc.vector.tensor_scalar(t2[:], pre[:], 0.0, None, OP.min)
                        nc.vector.tensor_tensor(
                            out=t2[:], in0=t2[:], in1=alphab_sb[:], op=OP.mult
                        )
                        nc.vector.tensor_tensor(
                            out=dst, in0=t1[:], in1=t2[:], op=OP.add
                        )

                nc.sync.dma_start(
                    out=out[:][ch * GB * P:(ch + 1) * GB * P, :]
                    .rearrange("(i p) f -> p i f", p=P),
                    in_=ob[:].rearrange("p (i f) -> p i f", i=GB),
                )
    nc.finalize()
    return nc


# ----------------------------------------------------------------------------
# Entry point
# ----------------------------------------------------------------------------

N_CORES = 8
TRACE = False          # set True (e.g. from test.py) to capture an NTFF profile
LAST_RESULT = None     # BassKernelResults of the most recent kernel() call


def _install_ntff_hook():
    """Provide antenv.axon_hooks if the image lacks it (needed for trace=True)."""
    import sys, types
    try:
        from antenv import axon_hooks  # noqa: F401
        return
    except ImportError:
        pass
    try:
        import antenv
        from trn_agent_boot.trn_boot import _ntff_profile_via_ctypes
        hook = [_ntff_profile_via_ctypes("/opt/axon/libaxon_pjrt.so")]
    except Exception:
        return
    mod = types.ModuleType("antenv.axon_hooks")
    mod.set_axon_ntff_profile_hook = lambda h: hook.__setitem__(0, h)
    mod.get_axon_ntff_profile_hook = lambda: hook[0]
    sys.modules["antenv.axon_hooks"] = mod
    antenv.axon_hooks = mod


def kernel(x, edge_index, W, b, alpha):
    global LAST_RESULT
    if TRACE:
        _install_ntff_hook()
    from concourse.bass_utils import run_bass_kernel_spmd

    cfg, shared, cores = _host_prep(x, edge_index, W, b, alpha, N_CORES)
    nc = _build_program(cfg)
    in_maps = []
    for c in range(N_CORES):
        m = dict(shared)
        m.update(cores[c])
        in_maps.append(m)
    res = run_bass_kernel_spmd(nc, in_maps, list(range(N_CORES)), trace=TRACE)
    LAST_RESULT = res
    shard = cfg["shard"]
    outs = [np.asarray(res.results[c]["out"])[:shard] for c in range(N_CORES)]
    return np.concatenate(outs, axis=0)


# revision 20
# speedup vs baseline: 5.6105x; 1.3846x over previous
"""GCN layer (PyG GCNConv + PReLU) as a Trainium2 Bass kernel, SPMD over 8 NeuronCores.

Math (matching the reference):
    deg[c]  = in_degree(c) + 1          (over edge destinations)
    dis     = deg ** -0.5
    x_pre   = dis[:,None] * x                           (bf16)
    aggF[:, c] = sum_{e: col_e = c} x_pre[row_e].T  + x_pre[c].T   (features x dests)
    out[c]  = PReLU( dis[c] * (aggF[:, c].T @ W) + b )

Per-core pipeline (dests sharded 12500/core; every core holds the full x_pre):
  * edges binned by (chunk of 7 dest blocks, src segment of 25088 rows), packed
    compactly (dest-sorted, blocks straddle tiles) with TRAILING -1 pads only -
    the dma_gather ucode strips trailing negatives, so padding costs no
    descriptor-generation time (the actual bottleneck: ~8ns/edge on the Pool
    engine's Q7 descriptor-gen loop).
  * one dma_gather per (chunk, seg) cell fetches x_pre[src] rows ([128e, 128f]
    bf16 tiles).
  * one-hot S tiles ([128e, 128d] bf16) for the segment-sum-as-matmul are built
    in ONE DVE instruction per (chunk, seg) via stride-0 broadcast APs:
    S[p, m, j] = (crel[p, m] == iota[j]).
  * per dest block: PSUM accumulates aggF = sum_m Xg_tile(m)^T-free x S(m) over
    the union (static across cores) matmul schedule; a DVE add folds in the
    self-loop term and copies PSUM->SBUF; a second matmul applies W; the DVE
    epilogue applies dis[c], bias, PReLU.
"""

import math
import numpy as np
import ml_dtypes

P = 128
D = 128
BF16 = ml_dtypes.bfloat16


# ----------------------------------------------------------------------------
# Host-side preparation (index/layout only - no per-edge feature data)
# ----------------------------------------------------------------------------

def _host_prep(x, edge_index, W, b, alpha, n_cores):
    x = np.asarray(x, dtype=np.float32)
    ei = np.asarray(edge_index)
    W = np.asarray(W, dtype=np.float32)
    b = np.asarray(b, dtype=np.float32)
    alpha = np.asarray(alpha, dtype=np.float32)
    n_nodes = x.shape[0]
    src, col = ei[0].astype(np.int64), ei[1].astype(np.int64)

    shard = n_nodes // n_cores
    assert shard * n_cores == n_nodes

    deg = (np.bincount(col, minlength=n_nodes) + 1.0).astype(np.float32)
    dis = (1.0 / np.sqrt(deg)).astype(np.float32)

    NSEG = 4
    n_src_pad = ((n_nodes + NSEG * 512 - 1) // (NSEG * 512)) * (NSEG * 512)
    seg_rows = n_src_pad // NSEG
    assert seg_rows <= 32768

    NB = math.ceil(shard / P)          # dest blocks per core (98)
    GB = 7                             # blocks per chunk
    NCH = math.ceil(NB / GB)           # chunks (14)
    NBP = NCH * GB                     # padded block count (98)

    xp = np.zeros((n_src_pad, D), dtype=BF16)
    xp[:n_nodes] = (x * dis[:, None]).astype(BF16)

    iota = np.broadcast_to(
        np.arange(P, dtype=np.float32), (P, P)).astype(BF16).copy()

    # ---- per-core binning ------------------------------------------------
    per_core = []
    T = 0
    for c in range(n_cores):
        lo = c * shard
        m = (col >= lo) & (col < lo + shard)
        s = src[m]
        dloc = col[m] - lo
        bi = dloc >> 7
        ch = bi // GB
        seg = s // seg_rows
        cell = ch * NSEG + seg
        order = np.argsort(cell * NBP + bi, kind="stable")
        s, dloc, bi, ch, seg, cell = (
            s[order], dloc[order], bi[order], ch[order], seg[order], cell[order])
        cnt = np.bincount(cell, minlength=NCH * NSEG)
        off = np.concatenate([[0], np.cumsum(cnt)])[:-1]
        slot = np.arange(len(s)) - off[cell]
        T = max(T, int(math.ceil(cnt.max() / P)))
        per_core.append(dict(s=s, dloc=dloc, bi=bi, cell=cell, slot=slot,
                             cnt=cnt, seg=seg))

    CIDX = T * P

    # ---- union (static) matmul schedule ----------------------------------
    # blockset[(ch, seg, t)] = union over cores of dest blocks present in tile t
    touch = set()
    for pc in per_core:
        cs = pc["cell"]
        t = pc["slot"] >> 7
        k = np.stack([cs // NSEG, cs % NSEG, t, pc["bi"]], axis=1)
        touch.update(map(tuple, np.unique(k, axis=0)))
    colidx = {}
    scol0 = np.zeros((NCH, NSEG), np.int64)    # first col of (ch, seg) S-chunk
    sncol = np.zeros((NCH, NSEG), np.int64)
    ncol = 0
    for ch in range(NCH):
        for seg in range(NSEG):
            scol0[ch, seg] = ncol
            for t in range(T):
                for bb in sorted(bi for (c2, s2, t2, bi) in touch
                                 if c2 == ch and s2 == seg and t2 == t):
                    colidx[(ch, seg, t, bb)] = ncol
                    ncol += 1
            sncol[ch, seg] = ncol - scol0[ch, seg]
    # per-block issue order (seg-major, then tile)
    sched = [[[] for _ in range(GB)] for _ in range(NCH)]
    for (ch, seg, t, bb), colv in sorted(colidx.items(), key=lambda kv: kv[1]):
        sched[ch][bb - ch * GB].append((seg, t, colv))
    for ch in range(NCH):
        for brel in range(GB):
            sched[ch][brel].sort(key=lambda x: (x[0], x[1]))
            assert len(sched[ch][brel]) > 0

    # ---- per-core tables -------------------------------------------------
    cores = []
    for c, pc in enumerate(per_core):
        cell, slot, s, seg = pc["cell"], pc["slot"], pc["s"], pc["seg"]
        # gather indices, wrapped 16 and replicated x8
        seq = np.full(NCH * NSEG * CIDX, -1, np.int16)
        j = cell * CIDX + slot
        seq[j] = (s - seg * seg_rows).astype(np.int16)
        table16 = np.zeros((16, len(seq) // 16), np.int16)
        # within each call the wrap is local: idx j of call q lives at
        # [(j % 16), q*CIDX/16 + j//16]
        q = np.arange(len(seq)) // CIDX
        r = np.arange(len(seq)) % CIDX
        table16[r % 16, q * (CIDX // 16) + r // 16] = seq
        gidx = np.tile(table16, (8, 1))

        crel = np.full((P, ncol), -1.0, np.float32)
        colv = np.array([colidx[(cl // NSEG, cl % NSEG, sl >> 7, bb)]
                         for cl, sl, bb in zip(cell, slot, pc["bi"])],
                        dtype=np.int64)
        crel[slot % P, colv] = (pc["dloc"] - (pc["bi"] << 7)).astype(np.float32)
        crel = crel.astype(BF16)

        own = np.minimum(c * shard + np.arange(NBP * P), n_nodes - 1)
        diso = dis[own.reshape(NBP, P).T].astype(np.float32)
        xpT = np.ascontiguousarray(
            (x[own] * dis[own, None]).T.astype(np.float32))
        cnts_v = pc["cnt"].astype(np.int32).reshape(1, NCH * NSEG)
        cores.append(dict(gidx=gidx, crel=crel, diso=diso, xpT=xpT,
                          cnts=cnts_v))

    cfg = dict(
        n_src_pad=n_src_pad, seg_rows=seg_rows, nb=NB, nbp=NBP, gb=GB,
        nch=NCH, n_segs=NSEG, T=T, cidx=CIDX, ncol=ncol,
        scol0=scol0.tolist(), sncol=sncol.tolist(), sched=sched,
        shard=shard,
        uniform_alpha=bool(np.ptp(alpha) == 0.0),
        alpha0=float(alpha.flat[0]),
        has_bias=bool(np.any(b != 0.0)),
    )
    alphab = np.broadcast_to(alpha, (P, D)).copy()
    biasb = np.broadcast_to(b, (P, D)).copy()
    shared = dict(W=W, iota=iota, alphab=alphab, biasb=biasb)
    for s in range(NSEG):
        shared[f"xp{s}"] = np.ascontiguousarray(
            xp[s * seg_rows:(s + 1) * seg_rows])
    return cfg, shared, cores


# ----------------------------------------------------------------------------
# Device program
# ----------------------------------------------------------------------------

DBG_NCH = None       # debug: limit chunk count
DBG_NOMM = False     # debug: skip matmul chains
DBG_NOGATHER = False  # debug: skip gathers


def _build_program(cfg):
    import concourse.bass as bass
    import concourse.bacc as bacc
    import concourse.mybir as mybir
    import concourse.tile as tile
    from contextlib import ExitStack

    f32 = mybir.dt.float32
    bf16 = mybir.dt.bfloat16
    i16 = mybir.dt.int16
    OP = mybir.AluOpType
    AF = mybir.ActivationFunctionType

    NSP = cfg["n_src_pad"]
    SEGR = cfg["seg_rows"]
    NBP = cfg["nbp"]
    GB = cfg["gb"]
    NCH = cfg["nch"]
    NSEG = cfg["n_segs"]
    T = cfg["T"]
    CIDX = cfg["cidx"]
    NCOL = cfg["ncol"]
    scol0 = cfg["scol0"]
    sncol = cfg["sncol"]
    sched = cfg["sched"]
    TOT16 = NCH * NSEG * (CIDX // 16)

    nc = bacc.Bacc(num_swdge_queues=4)
    xps = [
        nc.declare_dram_parameter(f"xp{s}", [SEGR, D], bf16, isOutput=False)
        for s in range(NSEG)
    ]
    cnts = nc.declare_dram_parameter("cnts", [1, NCH * NSEG], mybir.dt.int32,
                                     isOutput=False)
    Wp = nc.declare_dram_parameter("W", [P, D], f32, isOutput=False)
    gidx = nc.declare_dram_parameter("gidx", [P, TOT16], i16, isOutput=False)
    crel = nc.declare_dram_parameter("crel", [P, NCOL], bf16, isOutput=False)
    iota = nc.declare_dram_parameter("iota", [P, P], bf16, isOutput=False)
    diso = nc.declare_dram_parameter("diso", [P, NBP], f32, isOutput=False)
    xpT = nc.declare_dram_parameter("xpT", [P, NBP * P], f32, isOutput=False)
    alphab = nc.declare_dram_parameter("alphab", [P, D], f32, isOutput=False)
    biasb = nc.declare_dram_parameter("biasb", [P, D], f32, isOutput=False)
    out = nc.declare_dram_parameter("out", [NBP * P, D], f32, isOutput=True)

    with tile.TileContext(nc) as tc, ExitStack() as ctx:
        const_p = ctx.enter_context(tc.tile_pool(name="const", bufs=1))
        W_sb = const_p.tile([P, D], f32)
        nc.sync.dma_start(out=W_sb[:], in_=Wp[:])
        iota_sb = const_p.tile([P, P], bf16)
        nc.sync.dma_start(out=iota_sb[:], in_=iota[:])
        diso_sb = const_p.tile([P, NBP], f32)
        nc.sync.dma_start(out=diso_sb[:], in_=diso[:])
        alphab_sb = const_p.tile([P, D], f32)
        nc.sync.dma_start(out=alphab_sb[:], in_=alphab[:])
        biasb_sb = const_p.tile([P, D], f32)
        nc.sync.dma_start(out=biasb_sb[:], in_=biasb[:])
        cnt_sb = const_p.tile([1, NCH * NSEG], mybir.dt.int32)
        nc.sync.dma_start(out=cnt_sb[:], in_=cnts[:])
        cnt_reg = nc.gpsimd.alloc_register("gather_cnt")

        with (
            tc.tile_pool(name="ix", bufs=2 * NSEG) as ix_p,
            tc.tile_pool(name="yg", bufs=2 * NSEG) as yg_p,
            tc.tile_pool(name="S", bufs=2 * NSEG) as s_p,
            tc.tile_pool(name="crl", bufs=2) as crl_p,
            tc.tile_pool(name="xpt", bufs=2) as xpt_p,
            tc.tile_pool(name="agg", bufs=3) as agg_p,
            tc.tile_pool(name="psA", bufs=2, space="PSUM") as psA_p,
            tc.tile_pool(name="psB", bufs=2, space="PSUM") as psB_p,
            tc.tile_pool(name="ob", bufs=2) as ob_p,
            tc.tile_pool(name="eps", bufs=3) as ep_p,
        ):
            nch_run = NCH if DBG_NCH is None else DBG_NCH
            for ch in range(nch_run):
                ygs, ss = [], []
                for seg in range(NSEG):
                    q = ch * NSEG + seg
                    ix = ix_p.tile([P, CIDX // 16], i16)
                    nc.sync.dma_start(
                        out=ix[:],
                        in_=gidx[:][:, q * (CIDX // 16):(q + 1) * (CIDX // 16)],
                    )
                    yg = yg_p.tile([P, T * P], bf16)
                    # Pad slots are never gathered (trailing -1 stripped by
                    # the ucode); they are masked by S==0 in the matmul, which
                    # is only NaN-safe if the stale bits are finite. Virgin
                    # SBUF isn't guaranteed finite, so zero each buffer on its
                    # first two uses; after that the slots hold old gathered
                    # x values (finite), and skipping the memset removes a
                    # DVE->Pool dependency that stalls the gather queue.
                    if ch < 2:
                        nc.vector.memset(yg[:], 0.0)
                    if not DBG_NOGATHER:
                        # num_idxs_reg must equal the per-core count of real
                        # (non-negative) indices: the decode's descriptor-ring
                        # bookkeeping uses the register while the ucode strips
                        # trailing -1 pads; a mismatch desyncs the ring.
                        nc.gpsimd.reg_load(cnt_reg, cnt_sb[0:1, q:q + 1])
                        nc.gpsimd.dma_gather(
                            out_ap=yg[:].rearrange("p (t f) -> p t f", f=P),
                            in_ap=xps[seg][:],
                            idxs_ap=ix[:],
                            num_idxs=CIDX,
                            num_idxs_reg=cnt_reg,
                            elem_size=D,
                            single_packet=False,
                            queue_num=seg,
                        )
                    ygs.append(yg)
                    nc0 = sncol[ch][seg]
                    crl = crl_p.tile([P, nc0], bf16, tag=f"crl{seg}")
                    nc.sync.dma_start(
                        out=crl[:],
                        in_=crel[:][:, scol0[ch][seg]:scol0[ch][seg] + nc0],
                    )
                    S = s_p.tile([P, nc0 * P], bf16)
                    nc.vector.tensor_tensor(
                        out=S[:].rearrange("p (m j) -> p m j", j=P),
                        in0=crl[:].unsqueeze(2).broadcast_to([P, nc0, P]),
                        in1=iota_sb[:].unsqueeze(1).broadcast_to([P, nc0, P]),
                        op=OP.is_equal,
                    )
                    ss.append(S)

                xpt = xpt_p.tile([P, GB * P], f32)
                nc.sync.dma_start(
                    out=xpt[:], in_=xpT[:][:, ch * GB * P:(ch + 1) * GB * P]
                )
                ob = ob_p.tile([P, GB * D], f32)

                for brel in range(GB):
                    b2 = ch * GB + brel
                    mlist = sched[ch][brel]
                    ps = psA_p.tile([P, P], f32)
                    if DBG_NOMM:
                        mlist = mlist[:1]
                    for k, (seg, t, colv) in enumerate(mlist):
                        lc = colv - scol0[ch][seg]
                        nc.tensor.matmul(
                            out=ps[:],
                            lhsT=ygs[seg][:, t * P:(t + 1) * P],
                            rhs=ss[seg][:, lc * P:(lc + 1) * P],
                            start=(k == 0), stop=(k == len(mlist) - 1),
                        )
                    # fold in self-loop term and move PSUM->SBUF (f32)
                    agg = agg_p.tile([P, P], f32)
                    nc.vector.tensor_tensor(
                        out=agg[:], in0=ps[:],
                        in1=xpt[:, brel * P:(brel + 1) * P], op=OP.add,
                    )
                    ps2 = psB_p.tile([P, D], f32)
                    nc.tensor.matmul(
                        out=ps2[:], lhsT=agg[:], rhs=W_sb[:],
                        start=True, stop=True,
                    )
                    # epilogue: z = dis[c] * ps2 (+ b); prelu
                    dst = ob[:, brel * D:(brel + 1) * D]
                    if cfg["uniform_alpha"] and not cfg["has_bias"]:
                        # one ACT instruction: Lrelu(ps2 * dis[c]); keeps the
                        # epilogue (and its dependency stall) off the DVE.
                        nc.scalar.activation(
                            dst, ps2[:], AF.Lrelu,
                            bias=0.0, scale=diso_sb[:, b2:b2 + 1],
                            alpha=cfg["alpha0"],
                        )
                    else:
                        pre = ep_p.tile([P, D], f32, tag="pre")
                        nc.vector.tensor_scalar(
                            pre[:], ps2[:], diso_sb[:, b2:b2 + 1], None, OP.mult
                        )
                        if cfg["has_bias"]:
                            nc.vector.tensor_tensor(
                                out=pre[:], in0=pre[:], in1=biasb_sb[:],
                                op=OP.add
                            )
                        t1 = ep_p.tile([P, D], f32, tag="t1")
                        nc.vector.tensor_scalar(t1[:], pre[:], 0.0, None, OP.max)
                        t2 = ep_p.tile([P, D], f32, tag="t2")
                        if cfg["uniform_alpha"]:
                            nc.vector.tensor_scalar(
                                t2[:], pre[:], 0.0, cfg["alpha0"],
                                OP.min, OP.mult
                            )
                        else:
                            nc.vector.tensor_scalar(
                                t2[:], pre[:], 0.0, None, OP.min)
                            nc.vector.tensor_tensor(
                                out=t2[:], in0=t2[:], in1=alphab_sb[:],
                                op=OP.mult
                            )
                        nc.vector.tensor_tensor(
                            out=dst, in0=t1[:], in1=t2[:], op=OP.add
                        )

                nc.sync.dma_start(
                    out=out[:][ch * GB * P:(ch + 1) * GB * P, :]
                    .rearrange("(i p) f -> p i f", p=P),
                    in_=ob[:].rearrange("p (i f) -> p i f", i=GB),
                )
    nc.finalize()
    return nc


# ----------------------------------------------------------------------------
# Entry point
# ----------------------------------------------------------------------------

N_CORES = 8
TRACE = False          # set True (e.g. from test.py) to capture an NTFF profile
LAST_RESULT = None     # BassKernelResults of the most recent kernel() call


def _install_ntff_hook():
    """Provide antenv.axon_hooks if the image lacks it (needed for trace=True)."""
    import sys, types
    try:
        from antenv import axon_hooks  # noqa: F401
        return
    except ImportError:
        pass
    try:
        import antenv
        from trn_agent_boot.trn_boot import _ntff_profile_via_ctypes
        hook = [_ntff_profile_via_ctypes("/opt/axon/libaxon_pjrt.so")]
    except Exception:
        return
    mod = types.ModuleType("antenv.axon_hooks")
    mod.set_axon_ntff_profile_hook = lambda h: hook.__setitem__(0, h)
    mod.get_axon_ntff_profile_hook = lambda: hook[0]
    sys.modules["antenv.axon_hooks"] = mod
    antenv.axon_hooks = mod


def kernel(x, edge_index, W, b, alpha):
    global LAST_RESULT
    if TRACE:
        _install_ntff_hook()
    from concourse.bass_utils import run_bass_kernel_spmd

    cfg, shared, cores = _host_prep(x, edge_index, W, b, alpha, N_CORES)
    nc = _build_program(cfg)
    in_maps = []
    for c in range(N_CORES):
        m = dict(shared)
        m.update(cores[c])
        in_maps.append(m)
    res = run_bass_kernel_spmd(nc, in_maps, list(range(N_CORES)), trace=TRACE)
    LAST_RESULT = res
    shard = cfg["shard"]
    outs = [np.asarray(res.results[c]["out"])[:shard] for c in range(N_CORES)]
    return np.concatenate(outs, axis=0)
